# revision 20
# baseline (speedup 1.0000x reference)
"""CavemanGPT single-head attention on 8 Trainium2 NeuronCores.

Math (reference; its mask input is unused there):
    Q = emb @ W_q^T ; K = emb @ W_k^T ; V = emb @ W_v^T        (per batch b)
    out = softmax(K @ Q^T / sqrt(H), axis=-1) @ V

Key algebraic restructure: K @ Q^T = emb @ (W_k^T W_q) @ emb^T, so with
G := W_k^T @ W_q  ([E, E], batch independent) the per-core work drops from
~52 GFLOP to ~16 GFLOP and the giant [S, H] Q/K intermediates vanish:
    AT := (G^T @ emb_i^T) / 64     ([E, SI])
    scores = AT^T @ emb^T          ([SI, S], = true scores / 2)
    out = softmax(...) @ V

Two launches:
  1. G-launch: G = W_k^T @ W_q sharded over 8 cores (2 e'-halves x 4
     h-quarters); host sums the h-partials (in fp64).
  2. Main launch: 8 cores = 4 batches x 2 halves of the i (output-row)
     dimension. Each core receives its batch's emb with its own i-half
     permuted to the front (softmax over j is permutation invariant) and
     produces out[i-half].

Precision: the scores chain needs ~fp32 accuracy (softmax here is a
near-argmax; top-2 score gaps go down to ~0.06 while |scores| reaches 1.7e5),
but fp32 matmuls run at ~3.5 cyc/row on the PE and fp32r at ~2.25. fp16
streams at 1 cyc/row, so every chain tensor x is held as a hi/lo fp16 pair
(x = xh + xl, 11+11 mantissa bits) and each product uses 3 full-rate
matmuls: Ah*Bh + Ah*Bl + Al*Bh, accumulated in fp32 PSUM -- fp32-grade
products at ~3x fp16 speed. Inputs are pre-scaled by powers of two
(emb*32, W*32, AT/64) so the lo limbs stay in fp16 normal range; the exact
compensation happens in PSUM-evacuation scales and the softmax exp scale.
V and the attn@V stage are post-softmax (error passes through linearly) and
use single fp16.
"""

import math

import numpy as np

import concourse.bass as bass
import concourse.bass_utils as _bu
import concourse.mybir as mybir
import concourse.tile as tile
from concourse import bacc
from concourse.bass_utils import run_bass_kernel_spmd
from concourse.masks import make_identity

# LDWEIGHTS dedup: consecutive matmuls sharing a stationary operand skip the
# reload. Verified to produce bit-identical output on this kernel.
if not getattr(_bu, "_ldw_opt_patched", False):
    _orig_walrus_args = _bu.get_walrus_args

    def _walrus_args_ldw(arch, tmpdir, *, dve_root=None):
        args = _orig_walrus_args(arch, tmpdir, dve_root=dve_root)
        return [a.replace("--enable-ldw-opt=false", "--enable-ldw-opt=true") for a in args]

    _bu.get_walrus_args = _walrus_args_ldw
    _bu._ldw_opt_patched = True

dt = mybir.dt
P = 128
N_CORES = 8


def _split16(x):
    """x (fp32) -> (hi, lo) fp16 limbs with x ~= hi + lo (22-bit mantissa)."""
    x = np.ascontiguousarray(x, dtype=np.float32)
    hi = x.astype(np.float16)
    lo = (x - hi.astype(np.float32)).astype(np.float16)
    return hi, lo


def build_g_nc(S, E, H, O):
    """Launch 1: per-core partial G' = (32*W_k[hq])^T @ (32*W_q[hq][:, e'half])
    plus one (batch, j-half) shard of V = embT^T @ WvT (single fp16).

    Core c handles G e'-half (c % 2) / h-quarter (c // 2), and V for batch
    (c // 2), j-half (c % 2). Host sums the G h-partials and reassembles V.
    """
    SI = S // 2
    EH = E // 2
    HQ = H // 4
    EB = E // P
    HCB = HQ // P
    JBH = SI // P
    GW = min(512, EH)
    NGB = EH // GW
    OW = min(512, O)
    NOW = O // OW
    f32, f16 = dt.float32, dt.float16

    nc = bacc.Bacc("TRN2", target_bir_lowering=False, debug=False)
    wkh = nc.dram_tensor("wkh", [HQ, E], f16, kind="ExternalInput").ap()
    wkl = nc.dram_tensor("wkl", [HQ, E], f16, kind="ExternalInput").ap()
    wqh = nc.dram_tensor("wqh", [HQ, EH], f16, kind="ExternalInput").ap()
    wql = nc.dram_tensor("wql", [HQ, EH], f16, kind="ExternalInput").ap()
    evt = nc.dram_tensor("evt", [E, SI], f16, kind="ExternalInput").ap()
    wvt = nc.dram_tensor("wvt", [E, O], f16, kind="ExternalInput").ap()
    g_part = nc.dram_tensor("g_part", [E, EH], f32, kind="ExternalOutput").ap()
    v_part = nc.dram_tensor("v_part", [SI, O], f16, kind="ExternalOutput").ap()

    with tile.TileContext(nc) as tc:
        with (
            tc.tile_pool(name="p_res", bufs=1) as p_res,
            tc.tile_pool(name="p_vo", bufs=2) as p_vo,
            tc.tile_pool(name="p_gs", bufs=3) as p_gs,
            tc.tile_pool(name="ps_g", bufs=8, space="PSUM") as ps_g,
        ):
            # ---- G partial ----
            gp = p_res.tile([P, EB, EH], f32)
            evc = p_res.tile([P, EB, SI], f16)
            wvc = p_res.tile([P, EB, O], f16)
            pt_g = [
                [
                    ps_g.tile([P, GW], f32, tag="gps", name=f"gps_{eb}_{nb}")
                    for nb in range(NGB)
                ]
                for eb in range(EB)
            ]
            for hc in range(HCB):
                hs = slice(hc * P, (hc + 1) * P)
                kh = p_gs.tile([P, E], f16, tag="kh")
                nc.sync.dma_start(kh[:], wkh[hs, :])
                kl = p_gs.tile([P, E], f16, tag="kl")
                nc.sync.dma_start(kl[:], wkl[hs, :])
                qh = p_gs.tile([P, EH], f16, tag="qh")
                nc.sync.dma_start(qh[:], wqh[hs, :])
                ql = p_gs.tile([P, EH], f16, tag="ql")
                nc.sync.dma_start(ql[:], wql[hs, :])
                first, last = hc == 0, hc == HCB - 1
                for eb in range(EB):
                    ksl = slice(eb * P, (eb + 1) * P)
                    for nb in range(NGB):
                        nc.tensor.matmul(
                            pt_g[eb][nb][:], kh[:, ksl],
                            qh[:, nb * GW : (nb + 1) * GW], start=first, stop=False,
                        )
                    for nb in range(NGB):
                        nc.tensor.matmul(
                            pt_g[eb][nb][:], kh[:, ksl],
                            ql[:, nb * GW : (nb + 1) * GW], start=False, stop=False,
                        )
                    for nb in range(NGB):
                        nc.tensor.matmul(
                            pt_g[eb][nb][:], kl[:, ksl],
                            qh[:, nb * GW : (nb + 1) * GW], start=False, stop=last,
                        )
            gpr = g_part.rearrange("(eo p) e2 -> p eo e2", p=P)
            for eb in range(EB):
                for nb in range(NGB):
                    nsl = slice(nb * GW, (nb + 1) * GW)
                    nc.vector.tensor_scalar_mul(
                        gp[:, eb, nsl], pt_g[eb][nb][:], 2.0**-10
                    )
                # overlap the writeback with the remaining evacuations
                nc.sync.dma_start(gpr[:, eb], gp[:, eb])

            # ---- V shard (PE runs it after G; inputs loaded during G) ----
            nc.sync.dma_start(evc[:], evt.rearrange("(eo p) j -> p eo j", p=P))
            nc.sync.dma_start(wvc[:], wvt.rearrange("(eo p) o -> p eo o", p=P))
            for jb in range(JBH):
                jsl = slice(jb * P, (jb + 1) * P)
                pv_tiles = [
                    ps_g.tile([P, OW], f32, tag="gps", name=f"vps_{jb}_{ob}")
                    for ob in range(NOW)
                ]
                for eb in range(EB):
                    for ob in range(NOW):
                        osl = slice(ob * OW, (ob + 1) * OW)
                        nc.tensor.matmul(
                            pv_tiles[ob][:], evc[:, eb, jsl], wvc[:, eb, osl],
                            start=(eb == 0), stop=(eb == EB - 1),
                        )
                vt = p_vo.tile([P, O], f16, tag="vt")
                for ob in range(NOW):
                    osl = slice(ob * OW, (ob + 1) * OW)
                    nc.vector.tensor_scalar_mul(vt[:, osl], pv_tiles[ob][:], 2.0**-5)
                nc.sync.dma_start(v_part[jsl, :], vt[:])

    nc.compile()
    return nc


def build_main_nc(S, E, H, O):
    """Launch 2: attention for one (batch, i-half); G given as fp16 limbs."""
    SI = S // 2          # i rows per core
    EB = E // P          # 128-chunks of the embedding dim
    JB = S // P
    IB = SI // P
    IW = min(512, SI)    # AT moving width along i
    NIH = SI // IW
    JW = min(512, S)     # scores moving width along j
    NJW = S // JW
    OW = min(512, O)
    NOW = O // OW
    # scores PSUM = (AT/64)*(emb*32) = raw/2 ; exp arg must be raw/sqrt(H)
    SCALE_EXP = 2.0 / math.sqrt(H)

    f32, f16 = dt.float32, dt.float16

    nc = bacc.Bacc("TRN2", target_bir_lowering=False, debug=False)
    g_h = nc.dram_tensor("g_h", [E, E], f16, kind="ExternalInput").ap()
    g_l = nc.dram_tensor("g_l", [E, E], f16, kind="ExternalInput").ap()
    et_h = nc.dram_tensor("et_h", [E, S], f16, kind="ExternalInput").ap()
    et_l = nc.dram_tensor("et_l", [E, S], f16, kind="ExternalInput").ap()
    v_in = nc.dram_tensor("v_in", [S, O], f16, kind="ExternalInput").ap()
    out = nc.dram_tensor("out", [SI, O], f32, kind="ExternalOutput").ap()

    with tile.TileContext(nc) as tc:
        with (
            tc.tile_pool(name="misc", bufs=2) as misc,
            tc.tile_pool(name="p_big", bufs=1) as p_big,
        ):
            ident = misc.tile([P, P], f16, tag="ident", name="ident")
            make_identity(nc, ident[:])

            # whole-kernel residents
            eth = p_big.tile([P, EB, S], f16)   # embT*32 hi: [e part, e chunk, tok]
            etl = p_big.tile([P, EB, S], f16)
            ath = p_big.tile([P, EB, SI], f16)  # AT/64: [e' part, e' chunk, i]
            atl = p_big.tile([P, EB, SI], f16)
            v16 = p_big.tile([P, JB, O], f16)   # V: [j part, j chunk, o]


            with tc.tile_pool(name="ps", bufs=8, space="PSUM") as ps:
                # ---- AT = G^T embT / 64 (hi/lo split x3) ----
                with tc.tile_pool(name="p_g", bufs=1) as p_g:
                    gh = p_g.tile([P, EB, E], f16)  # [e part, e chunk, e']
                    gl = p_g.tile([P, EB, E], f16)
                    # DMAs emitted in first-use order, chunked per e-block so
                    # the first AT matmuls start after ~384KB instead of 14MB.
                    ghr = g_h.rearrange("(eo p) e2 -> p eo e2", p=P)
                    glr = g_l.rearrange("(eo p) e2 -> p eo e2", p=P)
                    ethr = et_h.rearrange("(eo p) t -> p eo t", p=P)
                    etlr = et_l.rearrange("(eo p) t -> p eo t", p=P)
                    for eb in range(EB):
                        nc.sync.dma_start(gh[:, eb], ghr[:, eb])
                        nc.sync.dma_start(eth[:, eb, :SI], ethr[:, eb, :SI])
                        nc.sync.dma_start(gl[:, eb], glr[:, eb])
                        nc.sync.dma_start(etl[:, eb, :SI], etlr[:, eb, :SI])
                    if SI < S:
                        nc.sync.dma_start(eth[:, :, SI:], ethr[:, :, SI:])
                        nc.sync.dma_start(etl[:, :, SI:], etlr[:, :, SI:])
                    nc.sync.dma_start(
                        v16[:], v_in.rearrange("(jo p) o -> p jo o", p=P)
                    )
                    for ih in range(NIH):
                        isl = slice(ih * IW, (ih + 1) * IW)
                        pts = [
                            ps.tile([P, IW], f32, tag="ps", name=f"aps_{ih}_{epb}")
                            for epb in range(EB)
                        ]
                        for eb in range(EB):
                            first, last = eb == 0, eb == EB - 1
                            for epb in range(EB):
                                psl = slice(epb * P, (epb + 1) * P)
                                pt = pts[epb]
                                nc.tensor.matmul(
                                    pt[:], gh[:, eb, psl], eth[:, eb, isl],
                                    start=first, stop=False,
                                )
                                nc.tensor.matmul(
                                    pt[:], gh[:, eb, psl], etl[:, eb, isl],
                                    start=False, stop=False,
                                )
                                nc.tensor.matmul(
                                    pt[:], gl[:, eb, psl], eth[:, eb, isl],
                                    start=False, stop=last,
                                )
                        for epb in range(EB):
                            psl = slice(epb * P, (epb + 1) * P)
                            pt = pts[epb]
                            atmp = misc.tile([P, IW], f32, tag="atmp", name=f"atmp_{ih}_{epb}")
                            nc.vector.tensor_scalar_mul(atmp[:], pt[:], 2.0**-11)
                            nc.vector.tensor_copy(ath[:, epb, isl], atmp[:])
                            nc.vector.tensor_tensor(
                                atl[:, epb, isl], atmp[:], ath[:, epb, isl],
                                mybir.AluOpType.subtract,
                            )

                # ---- scores + softmax + out, fused per 128-row i block ----
                with (
                    tc.tile_pool(name="p_sw", bufs=2) as p_sw,
                    tc.tile_pool(name="p_sw1", bufs=2) as p_sw1,
                ):
                    def emit_scores(ib):
                        ibs = slice(ib * P, (ib + 1) * P)
                        pt_s = [
                            ps.tile([P, JW], f32, tag="ps", name=f"sps_{ib}_{w}")
                            for w in range(NJW)
                        ]
                        for epb in range(EB):
                            for w in range(NJW):
                                wsl = slice(w * JW, (w + 1) * JW)
                                nc.tensor.matmul(
                                    pt_s[w][:], ath[:, epb, ibs], eth[:, epb, wsl],
                                    start=(epb == 0), stop=False,
                                )
                                nc.tensor.matmul(
                                    pt_s[w][:], ath[:, epb, ibs], etl[:, epb, wsl],
                                    start=False, stop=False,
                                )
                                nc.tensor.matmul(
                                    pt_s[w][:], atl[:, epb, ibs], eth[:, epb, wsl],
                                    start=False, stop=(epb == EB - 1),
                                )
                        return pt_s

                    pt_s = emit_scores(0)
                    for ib in range(IB):
                        ibs = slice(ib * P, (ib + 1) * P)
                        # two-stage row max straight off PSUM
                        mx4 = p_sw.tile([P, NJW], f32, tag="mx4")
                        for w in range(NJW):
                            nc.vector.reduce_max(
                                mx4[:, w : w + 1], pt_s[w][:], axis=mybir.AxisListType.X
                            )
                        nmx = p_sw.tile([P, 1], f32, tag="nmx")
                        nc.vector.reduce_max(
                            nmx[:], mx4[:], axis=mybir.AxisListType.X, negate=True
                        )
                        nmx2 = p_sw.tile([P, 1], f32, tag="nmx2")
                        nc.vector.tensor_scalar_mul(nmx2[:], nmx[:], SCALE_EXP)
                        # unnormalized exp, fp16, straight off PSUM; normalization
                        # is deferred to the output evacuation (x 1/sum per i-row)
                        attn16 = p_sw.tile([P, S], f16, tag="attn16")
                        for w in range(NJW):
                            nc.scalar.activation(
                                attn16[:, w * JW : (w + 1) * JW], pt_s[w][:],
                                mybir.ActivationFunctionType.Exp,
                                bias=nmx2[:], scale=SCALE_EXP,
                            )
                        sm = p_sw.tile([P, 1], f32, tag="sm")
                        nc.vector.reduce_sum(sm[:], attn16[:], axis=mybir.AxisListType.X)
                        rs = p_sw.tile([P, 1], f32, tag="rs")
                        nc.vector.reciprocal(rs[:], sm[:])
                        if ib + 1 < IB:
                            pt_s = emit_scores(ib + 1)
                        attnT = p_sw1.tile([P, JB, P], f16, tag="attnT")
                        for jb in range(JB):
                            tp = ps.tile([P, P], f16, tag="ps", name=f"tps_{ib}_{jb}")
                            nc.tensor.transpose(
                                tp[:], attn16[:, jb * P : (jb + 1) * P], ident[:]
                            )
                            nc.vector.tensor_copy(attnT[:, jb, :], tp[:])
                        pt_o = [
                            ps.tile([P, OW], f32, tag="ps", name=f"ops_{ib}_{ob}")
                            for ob in range(NOW)
                        ]
                        for jb in range(JB):
                            for ob in range(NOW):
                                nc.tensor.matmul(
                                    pt_o[ob][:],
                                    attnT[:, jb, :],
                                    v16[:, jb, ob * OW : (ob + 1) * OW],
                                    start=(jb == 0), stop=(jb == JB - 1),
                                )
                        outt = p_sw1.tile([P, O], f32, tag="outt")
                        for ob in range(NOW):
                            nc.vector.tensor_scalar_mul(
                                outt[:, ob * OW : (ob + 1) * OW], pt_o[ob][:], rs[:]
                            )
                        nc.sync.dma_start(out[ibs, :], outt[:])

    nc.compile()
    return nc


_NC_CACHE = {}


def _get_nc(builder, *key):
    k = (builder.__name__,) + key
    if k not in _NC_CACHE:
        _NC_CACHE[k] = builder(*key)
    return _NC_CACHE[k]


def kernel(token_emb, W_q, W_k, W_v, mask=None, _trace=False, _tmpdir=None):
    token_emb = np.asarray(token_emb, np.float32)
    W_q = np.asarray(W_q, np.float32)
    W_k = np.asarray(W_k, np.float32)
    W_v = np.asarray(W_v, np.float32)
    B, S, E = token_emb.shape
    H = W_q.shape[0]
    O = W_v.shape[0]
    SI = S // 2
    EH = E // 2
    HQ = H // 4
    assert 2 * B == N_CORES

    # ---- launch 1: sharded G = W_k^T @ W_q and V = emb @ W_v^T ----
    nc_g = _get_nc(build_g_nc, S, E, H, O)
    wk_h, wk_l = _split16(W_k * 32.0)
    wq_h, wq_l = _split16(W_q * 32.0)
    wvt = np.ascontiguousarray(W_v.T).astype(np.float16)
    emb_h = [
        _split16(np.ascontiguousarray(token_emb[b].T) * 32.0)[0] for b in range(B)
    ]
    g_maps = []
    for c in range(N_CORES):
        half, hq = c % 2, c // 2
        hsl = slice(hq * HQ, (hq + 1) * HQ)
        esl = slice(half * EH, (half + 1) * EH)
        b, jhalf = c // 2, c % 2
        g_maps.append(
            {
                "wkh": np.ascontiguousarray(wk_h[hsl]),
                "wkl": np.ascontiguousarray(wk_l[hsl]),
                "wqh": np.ascontiguousarray(wq_h[hsl, esl]),
                "wql": np.ascontiguousarray(wq_l[hsl, esl]),
                "evt": np.ascontiguousarray(emb_h[b][:, jhalf * SI : (jhalf + 1) * SI]),
                "wvt": wvt,
            }
        )
    res_g = run_bass_kernel_spmd(
        nc_g, g_maps, core_ids=list(range(N_CORES)), trace=_trace,
        tmpdir=(_tmpdir + "/g" if _tmpdir else None),
    )
    G = np.empty((E, E), np.float32)
    for half in range(2):
        esl = slice(half * EH, (half + 1) * EH)
        G[:, esl] = sum(
            res_g.results[2 * q + half]["g_part"].astype(np.float64)
            for q in range(4)
        ).astype(np.float32)
    g_h, g_l = _split16(G)
    v_nat = [
        np.concatenate(
            [res_g.results[2 * b + 0]["v_part"], res_g.results[2 * b + 1]["v_part"]],
            axis=0,
        )
        for b in range(B)
    ]

    # ---- launch 2: attention ----
    nc_main = _get_nc(build_main_nc, S, E, H, O)
    in_maps = []
    for c in range(N_CORES):
        b, half = divmod(c, 2)
        e = token_emb[b]
        perm = np.concatenate(
            [e[half * SI : (half + 1) * SI], e[(1 - half) * SI : (2 - half) * SI]],
            axis=0,
        )
        et_h, et_l = _split16(perm.T * 32.0)
        vp = v_nat[b]
        v_in = np.concatenate(
            [vp[half * SI : (half + 1) * SI], vp[(1 - half) * SI : (2 - half) * SI]],
            axis=0,
        )
        in_maps.append(
            {
                "g_h": g_h, "g_l": g_l, "et_h": et_h, "et_l": et_l,
                "v_in": np.ascontiguousarray(v_in),
            }
        )
    res = run_bass_kernel_spmd(
        nc_main, in_maps, core_ids=list(range(N_CORES)), trace=_trace,
        tmpdir=(_tmpdir + "/main" if _tmpdir else None),
    )

    out = np.empty((B, S, O), np.float32)
    for c in range(N_CORES):
        b, half = divmod(c, 2)
        out[b, half * SI : (half + 1) * SI] = res.results[c]["out"]
    if _trace:
        kernel._last_results = (res_g, res)
    return out


# revision 21
# speedup vs baseline: 1.0085x; 1.0085x over previous
"""CavemanGPT single-head attention on 8 Trainium2 NeuronCores.

Math (reference; its mask input is unused there):
    Q = emb @ W_q^T ; K = emb @ W_k^T ; V = emb @ W_v^T        (per batch b)
    out = softmax(K @ Q^T / sqrt(H), axis=-1) @ V

Key algebraic restructure: K @ Q^T = emb @ (W_k^T W_q) @ emb^T, so with
G := W_k^T @ W_q  ([E, E], batch independent) the per-core work drops from
~52 GFLOP to ~16 GFLOP and the giant [S, H] Q/K intermediates vanish:
    AT := (G^T @ emb_i^T) / 64     ([E, SI])
    scores = AT^T @ emb^T          ([SI, S], = true scores / 2)
    out = softmax(...) @ V

Two launches:
  1. G-launch: G = W_k^T @ W_q sharded over 8 cores (2 e'-halves x 4
     h-quarters); host sums the h-partials (in fp64).
  2. Main launch: 8 cores = 4 batches x 2 halves of the i (output-row)
     dimension. Each core receives its batch's emb with its own i-half
     permuted to the front (softmax over j is permutation invariant) and
     produces out[i-half].

Precision: the scores chain needs ~fp32 accuracy (softmax here is a
near-argmax; top-2 score gaps go down to ~0.06 while |scores| reaches 1.7e5),
but fp32 matmuls run at ~3.5 cyc/row on the PE and fp32r at ~2.25. fp16
streams at 1 cyc/row, so every chain tensor x is held as a hi/lo fp16 pair
(x = xh + xl, 11+11 mantissa bits) and each product uses 3 full-rate
matmuls: Ah*Bh + Ah*Bl + Al*Bh, accumulated in fp32 PSUM -- fp32-grade
products at ~3x fp16 speed. Inputs are pre-scaled by powers of two
(emb*32, W*32, AT/64) so the lo limbs stay in fp16 normal range; the exact
compensation happens in PSUM-evacuation scales and the softmax exp scale.
V and the attn@V stage are post-softmax (error passes through linearly) and
use single fp16.
"""

import math

import numpy as np

import concourse.bass as bass
import concourse.bass_utils as _bu
import concourse.mybir as mybir
import concourse.tile as tile
from concourse import bacc
from concourse.bass_utils import run_bass_kernel_spmd
from concourse.masks import make_identity

# LDWEIGHTS dedup: consecutive matmuls sharing a stationary operand skip the
# reload. Verified to produce bit-identical output on this kernel.
if not getattr(_bu, "_ldw_opt_patched", False):
    _orig_walrus_args = _bu.get_walrus_args

    def _walrus_args_ldw(arch, tmpdir, *, dve_root=None):
        args = _orig_walrus_args(arch, tmpdir, dve_root=dve_root)
        return [a.replace("--enable-ldw-opt=false", "--enable-ldw-opt=true") for a in args]

    _bu.get_walrus_args = _walrus_args_ldw
    _bu._ldw_opt_patched = True

dt = mybir.dt
P = 128
N_CORES = 8


def _split16(x):
    """x (fp32) -> (hi, lo) fp16 limbs with x ~= hi + lo (22-bit mantissa)."""
    x = np.ascontiguousarray(x, dtype=np.float32)
    hi = x.astype(np.float16)
    lo = (x - hi.astype(np.float32)).astype(np.float16)
    return hi, lo


def build_g_nc(S, E, H, O):
    """Launch 1: per-core partial G' = (32*W_k[hq])^T @ (32*W_q[hq][:, e'half])
    plus one (batch, j-half) shard of V = embT^T @ WvT (single fp16).

    Core c handles G e'-half (c % 2) / h-quarter (c // 2), and V for batch
    (c // 2), j-half (c % 2). Host sums the G h-partials and reassembles V.
    """
    SI = S // 2
    EH = E // 2
    HQ = H // 4
    EB = E // P
    HCB = HQ // P
    JBH = SI // P
    GW = min(512, EH)
    NGB = EH // GW
    OW = min(512, O)
    NOW = O // OW
    f32, f16 = dt.float32, dt.float16

    nc = bacc.Bacc("TRN2", target_bir_lowering=False, debug=False)
    wkh = nc.dram_tensor("wkh", [HQ, E], f16, kind="ExternalInput").ap()
    wkl = nc.dram_tensor("wkl", [HQ, E], f16, kind="ExternalInput").ap()
    wqh = nc.dram_tensor("wqh", [HQ, EH], f16, kind="ExternalInput").ap()
    wql = nc.dram_tensor("wql", [HQ, EH], f16, kind="ExternalInput").ap()
    evt = nc.dram_tensor("evt", [E, SI], f16, kind="ExternalInput").ap()
    wvt = nc.dram_tensor("wvt", [E, O], f16, kind="ExternalInput").ap()
    g_part = nc.dram_tensor("g_part", [E, EH], f32, kind="ExternalOutput").ap()
    v_part = nc.dram_tensor("v_part", [SI, O], f16, kind="ExternalOutput").ap()

    with tile.TileContext(nc) as tc:
        with (
            tc.tile_pool(name="p_res", bufs=1) as p_res,
            tc.tile_pool(name="p_vo", bufs=2) as p_vo,
            tc.tile_pool(name="p_gs", bufs=3) as p_gs,
            tc.tile_pool(name="ps_g", bufs=8, space="PSUM") as ps_g,
        ):
            # ---- G partial ----
            gp = p_res.tile([P, EB, EH], f32)
            evc = p_res.tile([P, EB, SI], f16)
            wvc = p_res.tile([P, EB, O], f16)
            pt_g = [
                [
                    ps_g.tile([P, GW], f32, tag="gps", name=f"gps_{eb}_{nb}")
                    for nb in range(NGB)
                ]
                for eb in range(EB)
            ]
            for hc in range(HCB):
                hs = slice(hc * P, (hc + 1) * P)
                # queue order matches first use: kh+qh feed the first matmul
                kh = p_gs.tile([P, E], f16, tag="kh")
                nc.sync.dma_start(kh[:], wkh[hs, :])
                qh = p_gs.tile([P, EH], f16, tag="qh")
                nc.sync.dma_start(qh[:], wqh[hs, :])
                ql = p_gs.tile([P, EH], f16, tag="ql")
                nc.sync.dma_start(ql[:], wql[hs, :])
                kl = p_gs.tile([P, E], f16, tag="kl")
                nc.sync.dma_start(kl[:], wkl[hs, :])
                first, last = hc == 0, hc == HCB - 1
                for eb in range(EB):
                    ksl = slice(eb * P, (eb + 1) * P)
                    for nb in range(NGB):
                        nc.tensor.matmul(
                            pt_g[eb][nb][:], kh[:, ksl],
                            qh[:, nb * GW : (nb + 1) * GW], start=first, stop=False,
                        )
                    for nb in range(NGB):
                        nc.tensor.matmul(
                            pt_g[eb][nb][:], kh[:, ksl],
                            ql[:, nb * GW : (nb + 1) * GW], start=False, stop=False,
                        )
                    for nb in range(NGB):
                        nc.tensor.matmul(
                            pt_g[eb][nb][:], kl[:, ksl],
                            qh[:, nb * GW : (nb + 1) * GW], start=False, stop=last,
                        )
            gpr = g_part.rearrange("(eo p) e2 -> p eo e2", p=P)
            for eb in range(EB):
                for nb in range(NGB):
                    nsl = slice(nb * GW, (nb + 1) * GW)
                    nc.vector.tensor_scalar_mul(
                        gp[:, eb, nsl], pt_g[eb][nb][:], 2.0**-10
                    )
                # overlap the writeback with the remaining evacuations
                nc.sync.dma_start(gpr[:, eb], gp[:, eb])

            # ---- V shard (PE runs it after G; inputs loaded during G) ----
            nc.sync.dma_start(evc[:], evt.rearrange("(eo p) j -> p eo j", p=P))
            nc.sync.dma_start(wvc[:], wvt.rearrange("(eo p) o -> p eo o", p=P))
            for jb in range(JBH):
                jsl = slice(jb * P, (jb + 1) * P)
                pv_tiles = [
                    ps_g.tile([P, OW], f32, tag="gps", name=f"vps_{jb}_{ob}")
                    for ob in range(NOW)
                ]
                for eb in range(EB):
                    for ob in range(NOW):
                        osl = slice(ob * OW, (ob + 1) * OW)
                        nc.tensor.matmul(
                            pv_tiles[ob][:], evc[:, eb, jsl], wvc[:, eb, osl],
                            start=(eb == 0), stop=(eb == EB - 1),
                        )
                vt = p_vo.tile([P, O], f16, tag="vt")
                for ob in range(NOW):
                    osl = slice(ob * OW, (ob + 1) * OW)
                    nc.vector.tensor_scalar_mul(vt[:, osl], pv_tiles[ob][:], 2.0**-5)
                nc.sync.dma_start(v_part[jsl, :], vt[:])

    nc.compile()
    return nc


def build_main_nc(S, E, H, O):
    """Launch 2: attention for one (batch, i-half); G given as fp16 limbs."""
    SI = S // 2          # i rows per core
    EB = E // P          # 128-chunks of the embedding dim
    JB = S // P
    IB = SI // P
    IW = min(512, SI)    # AT moving width along i
    NIH = SI // IW
    JW = min(512, S)     # scores moving width along j
    NJW = S // JW
    OW = min(512, O)
    NOW = O // OW
    # scores PSUM = (AT/64)*(emb*32) = raw/2 ; exp arg must be raw/sqrt(H)
    SCALE_EXP = 2.0 / math.sqrt(H)

    f32, f16 = dt.float32, dt.float16

    nc = bacc.Bacc("TRN2", target_bir_lowering=False, debug=False)
    g_h = nc.dram_tensor("g_h", [E, E], f16, kind="ExternalInput").ap()
    g_l = nc.dram_tensor("g_l", [E, E], f16, kind="ExternalInput").ap()
    et_h = nc.dram_tensor("et_h", [E, S], f16, kind="ExternalInput").ap()
    et_l = nc.dram_tensor("et_l", [E, S], f16, kind="ExternalInput").ap()
    v_in = nc.dram_tensor("v_in", [S, O], f16, kind="ExternalInput").ap()
    out = nc.dram_tensor("out", [SI, O], f32, kind="ExternalOutput").ap()

    with tile.TileContext(nc) as tc:
        with (
            tc.tile_pool(name="misc", bufs=2) as misc,
            tc.tile_pool(name="p_big", bufs=1) as p_big,
        ):
            ident = misc.tile([P, P], f16, tag="ident", name="ident")
            make_identity(nc, ident[:])

            # whole-kernel residents
            eth = p_big.tile([P, EB, S], f16)   # embT*32 hi: [e part, e chunk, tok]
            etl = p_big.tile([P, EB, S], f16)
            ath = p_big.tile([P, EB, SI], f16)  # AT/64: [e' part, e' chunk, i]
            atl = p_big.tile([P, EB, SI], f16)
            v16 = p_big.tile([P, JB, O], f16)   # V: [j part, j chunk, o]


            with tc.tile_pool(name="ps", bufs=8, space="PSUM") as ps:
                # ---- AT = G^T embT / 64 (hi/lo split x3) ----
                with tc.tile_pool(name="p_g", bufs=1) as p_g:
                    gh = p_g.tile([P, EB, E], f16)  # [e part, e chunk, e']
                    gl = p_g.tile([P, EB, E], f16)
                    # DMAs emitted in first-use order, chunked per e-block so
                    # the first AT matmuls start after ~384KB instead of 14MB.
                    ghr = g_h.rearrange("(eo p) e2 -> p eo e2", p=P)
                    glr = g_l.rearrange("(eo p) e2 -> p eo e2", p=P)
                    ethr = et_h.rearrange("(eo p) t -> p eo t", p=P)
                    etlr = et_l.rearrange("(eo p) t -> p eo t", p=P)
                    for eb in range(EB):
                        nc.sync.dma_start(gh[:, eb], ghr[:, eb])
                        nc.sync.dma_start(eth[:, eb, :SI], ethr[:, eb, :SI])
                        nc.sync.dma_start(gl[:, eb], glr[:, eb])
                        nc.sync.dma_start(etl[:, eb, :SI], etlr[:, eb, :SI])
                    if SI < S:
                        nc.sync.dma_start(eth[:, :, SI:], ethr[:, :, SI:])
                        nc.sync.dma_start(etl[:, :, SI:], etlr[:, :, SI:])
                    nc.sync.dma_start(
                        v16[:], v_in.rearrange("(jo p) o -> p jo o", p=P)
                    )
                    for ih in range(NIH):
                        isl = slice(ih * IW, (ih + 1) * IW)
                        pts = [
                            ps.tile([P, IW], f32, tag="ps", name=f"aps_{ih}_{epb}")
                            for epb in range(EB)
                        ]
                        for eb in range(EB):
                            first, last = eb == 0, eb == EB - 1
                            for epb in range(EB):
                                psl = slice(epb * P, (epb + 1) * P)
                                pt = pts[epb]
                                nc.tensor.matmul(
                                    pt[:], gh[:, eb, psl], eth[:, eb, isl],
                                    start=first, stop=False,
                                )
                                nc.tensor.matmul(
                                    pt[:], gh[:, eb, psl], etl[:, eb, isl],
                                    start=False, stop=False,
                                )
                                nc.tensor.matmul(
                                    pt[:], gl[:, eb, psl], eth[:, eb, isl],
                                    start=False, stop=last,
                                )
                        for epb in range(EB):
                            psl = slice(epb * P, (epb + 1) * P)
                            pt = pts[epb]
                            atmp = misc.tile([P, IW], f32, tag="atmp", name=f"atmp_{ih}_{epb}")
                            nc.vector.tensor_scalar_mul(atmp[:], pt[:], 2.0**-11)
                            nc.vector.tensor_copy(ath[:, epb, isl], atmp[:])
                            nc.vector.tensor_tensor(
                                atl[:, epb, isl], atmp[:], ath[:, epb, isl],
                                mybir.AluOpType.subtract,
                            )

                # ---- scores + softmax + out, fused per 128-row i block ----
                with (
                    tc.tile_pool(name="p_sw", bufs=2) as p_sw,
                    tc.tile_pool(name="p_sw1", bufs=2) as p_sw1,
                ):
                    def emit_scores(ib):
                        ibs = slice(ib * P, (ib + 1) * P)
                        pt_s = [
                            ps.tile([P, JW], f32, tag="ps", name=f"sps_{ib}_{w}")
                            for w in range(NJW)
                        ]
                        for epb in range(EB):
                            for w in range(NJW):
                                wsl = slice(w * JW, (w + 1) * JW)
                                nc.tensor.matmul(
                                    pt_s[w][:], ath[:, epb, ibs], eth[:, epb, wsl],
                                    start=(epb == 0), stop=False,
                                )
                                nc.tensor.matmul(
                                    pt_s[w][:], ath[:, epb, ibs], etl[:, epb, wsl],
                                    start=False, stop=False,
                                )
                                nc.tensor.matmul(
                                    pt_s[w][:], atl[:, epb, ibs], eth[:, epb, wsl],
                                    start=False, stop=(epb == EB - 1),
                                )
                        return pt_s

                    pt_s = emit_scores(0)
                    for ib in range(IB):
                        ibs = slice(ib * P, (ib + 1) * P)
                        # two-stage row max straight off PSUM
                        mx4 = p_sw.tile([P, NJW], f32, tag="mx4")
                        for w in range(NJW):
                            nc.vector.reduce_max(
                                mx4[:, w : w + 1], pt_s[w][:], axis=mybir.AxisListType.X
                            )
                        nmx = p_sw.tile([P, 1], f32, tag="nmx")
                        nc.vector.reduce_max(
                            nmx[:], mx4[:], axis=mybir.AxisListType.X, negate=True
                        )
                        nmx2 = p_sw.tile([P, 1], f32, tag="nmx2")
                        nc.vector.tensor_scalar_mul(nmx2[:], nmx[:], SCALE_EXP)
                        # unnormalized exp, fp16, straight off PSUM; normalization
                        # is deferred to the output evacuation (x 1/sum per i-row)
                        attn16 = p_sw.tile([P, S], f16, tag="attn16")
                        for w in range(NJW):
                            nc.scalar.activation(
                                attn16[:, w * JW : (w + 1) * JW], pt_s[w][:],
                                mybir.ActivationFunctionType.Exp,
                                bias=nmx2[:], scale=SCALE_EXP,
                            )
                        sm = p_sw.tile([P, 1], f32, tag="sm")
                        nc.vector.reduce_sum(sm[:], attn16[:], axis=mybir.AxisListType.X)
                        rs = p_sw.tile([P, 1], f32, tag="rs")
                        nc.vector.reciprocal(rs[:], sm[:])
                        if ib + 1 < IB:
                            pt_s = emit_scores(ib + 1)
                        attnT = p_sw1.tile([P, JB, P], f16, tag="attnT")
                        for jb in range(JB):
                            tp = ps.tile([P, P], f16, tag="ps", name=f"tps_{ib}_{jb}")
                            nc.tensor.transpose(
                                tp[:], attn16[:, jb * P : (jb + 1) * P], ident[:]
                            )
                            nc.vector.tensor_copy(attnT[:, jb, :], tp[:])
                        pt_o = [
                            ps.tile([P, OW], f32, tag="ps", name=f"ops_{ib}_{ob}")
                            for ob in range(NOW)
                        ]
                        for jb in range(JB):
                            for ob in range(NOW):
                                nc.tensor.matmul(
                                    pt_o[ob][:],
                                    attnT[:, jb, :],
                                    v16[:, jb, ob * OW : (ob + 1) * OW],
                                    start=(jb == 0), stop=(jb == JB - 1),
                                )
                        outt = p_sw1.tile([P, O], f32, tag="outt")
                        for ob in range(NOW):
                            nc.vector.tensor_scalar_mul(
                                outt[:, ob * OW : (ob + 1) * OW], pt_o[ob][:], rs[:]
                            )
                        nc.sync.dma_start(out[ibs, :], outt[:])

    nc.compile()
    return nc


_NC_CACHE = {}


def _get_nc(builder, *key):
    k = (builder.__name__,) + key
    if k not in _NC_CACHE:
        _NC_CACHE[k] = builder(*key)
    return _NC_CACHE[k]


def kernel(token_emb, W_q, W_k, W_v, mask=None, _trace=False, _tmpdir=None):
    token_emb = np.asarray(token_emb, np.float32)
    W_q = np.asarray(W_q, np.float32)
    W_k = np.asarray(W_k, np.float32)
    W_v = np.asarray(W_v, np.float32)
    B, S, E = token_emb.shape
    H = W_q.shape[0]
    O = W_v.shape[0]
    SI = S // 2
    EH = E // 2
    HQ = H // 4
    assert 2 * B == N_CORES

    # ---- launch 1: sharded G = W_k^T @ W_q and V = emb @ W_v^T ----
    nc_g = _get_nc(build_g_nc, S, E, H, O)
    wk_h, wk_l = _split16(W_k * 32.0)
    wq_h, wq_l = _split16(W_q * 32.0)
    wvt = np.ascontiguousarray(W_v.T).astype(np.float16)
    emb_h = [
        _split16(np.ascontiguousarray(token_emb[b].T) * 32.0)[0] for b in range(B)
    ]
    g_maps = []
    for c in range(N_CORES):
        half, hq = c % 2, c // 2
        hsl = slice(hq * HQ, (hq + 1) * HQ)
        esl = slice(half * EH, (half + 1) * EH)
        b, jhalf = c // 2, c % 2
        g_maps.append(
            {
                "wkh": np.ascontiguousarray(wk_h[hsl]),
                "wkl": np.ascontiguousarray(wk_l[hsl]),
                "wqh": np.ascontiguousarray(wq_h[hsl, esl]),
                "wql": np.ascontiguousarray(wq_l[hsl, esl]),
                "evt": np.ascontiguousarray(emb_h[b][:, jhalf * SI : (jhalf + 1) * SI]),
                "wvt": wvt,
            }
        )
    res_g = run_bass_kernel_spmd(
        nc_g, g_maps, core_ids=list(range(N_CORES)), trace=_trace,
        tmpdir=(_tmpdir + "/g" if _tmpdir else None),
    )
    G = np.empty((E, E), np.float32)
    for half in range(2):
        esl = slice(half * EH, (half + 1) * EH)
        G[:, esl] = sum(
            res_g.results[2 * q + half]["g_part"].astype(np.float64)
            for q in range(4)
        ).astype(np.float32)
    g_h, g_l = _split16(G)
    v_nat = [
        np.concatenate(
            [res_g.results[2 * b + 0]["v_part"], res_g.results[2 * b + 1]["v_part"]],
            axis=0,
        )
        for b in range(B)
    ]

    # ---- launch 2: attention ----
    nc_main = _get_nc(build_main_nc, S, E, H, O)
    in_maps = []
    for c in range(N_CORES):
        b, half = divmod(c, 2)
        e = token_emb[b]
        perm = np.concatenate(
            [e[half * SI : (half + 1) * SI], e[(1 - half) * SI : (2 - half) * SI]],
            axis=0,
        )
        et_h, et_l = _split16(perm.T * 32.0)
        vp = v_nat[b]
        v_in = np.concatenate(
            [vp[half * SI : (half + 1) * SI], vp[(1 - half) * SI : (2 - half) * SI]],
            axis=0,
        )
        in_maps.append(
            {
                "g_h": g_h, "g_l": g_l, "et_h": et_h, "et_l": et_l,
                "v_in": np.ascontiguousarray(v_in),
            }
        )
    res = run_bass_kernel_spmd(
        nc_main, in_maps, core_ids=list(range(N_CORES)), trace=_trace,
        tmpdir=(_tmpdir + "/main" if _tmpdir else None),
    )

    out = np.empty((B, S, O), np.float32)
    for c in range(N_CORES):
        b, half = divmod(c, 2)
        out[b, half * SI : (half + 1) * SI] = res.results[c]["out"]
    if _trace:
        kernel._last_results = (res_g, res)
    return out
